# revision 2
# baseline (speedup 1.0000x reference)
"""Trainium2 Bass kernel for the LSTM neighbor-aggregator GNN layer.

Strategy (all sizes hardcoded for N=30000, E=480000, D=H=128, max_deg=48):
- Nodes are sharded across 8 NeuronCores (data-parallel over nodes); the small
  LSTM / projection weights are replicated.
- On each core, node neighbor-sequences are bin-packed into 1024 column slots
  (8 granules x 128 columns) over a shared step timetable, so every LSTM step
  runs full-width [128, 1024] ops in a feature-transposed layout
  (hidden-unit on partitions, nodes on the free dim).
- Neighbor rows are fetched per step with dma_gather(transpose=True) from an
  fp16 copy of input_matrix, landing directly as x^T [128, 1024].
- Gates: per gate k, PSUM[128,1024] = W_ih_k @ x^T (fp16 matmuls)
  + W_hh_k @ h^T (fp32 matmuls). Sigmoid/tanh on the scalar engine with
  per-partition bias; cell math on the vector engine.
- Finished nodes' h columns are extracted each step with ap_gather into a
  staging buffer, compacted, then projected with W_out on-chip.
"""
import numpy as np
from contextlib import ExitStack

import concourse.bacc as bacc
import concourse.tile as tile
from concourse import mybir
from concourse.bass_utils import run_bass_kernel_spmd

N_NODES = 30000
N_EDGES = 480000
D = 128
HID = 128
MAX_DEG = 48
NCORES = 8
NGRAN = 8
GSIZE = 128
NCOL = NGRAN * GSIZE          # 1024
EXT_K = 128                   # extraction slots per step
NPER = N_NODES // NCORES      # 3750
NPROJ = ((NPER + 127) // 128) * 128  # 3840
PTILES = NPROJ // 128         # 30
F32 = mybir.dt.float32
F32R = mybir.dt.float32r
F16 = mybir.dt.float16
I16 = mybir.dt.int16

_CACHE = {}


# --------------------------------------------------------------------------
# host-side schedule
# --------------------------------------------------------------------------

def _build_schedule(edge_src, edge_trg):
    counts = np.bincount(edge_src, minlength=N_NODES)
    starts = np.cumsum(counts) - counts
    deg = np.minimum(counts, MAX_DEG).astype(np.int64)

    order = np.argsort(-deg, kind="stable")
    core_nodes = [order[c::NCORES] for c in range(NCORES)]
    queues = [nodes[deg[nodes] > 0] for nodes in core_nodes]
    iso = [nodes[deg[nodes] == 0] for nodes in core_nodes]

    next_free = [0] * NGRAN
    generations = []
    qpos = [0] * NCORES
    fin_counts = [dict() for _ in range(NCORES)]

    while any(qpos[c] < len(queues[c]) for c in range(NCORES)):
        g = int(np.argmin(next_free))
        s = next_free[g]
        gen_nodes = []
        L = 1
        for c in range(NCORES):
            take = list(queues[c][qpos[c]: qpos[c] + GSIZE])
            gen_nodes.append(take)
            if take:
                L = max(L, int(deg[take[0]]))
        while True:
            ok = True
            for c in range(NCORES):
                fc = {}
                for nd in gen_nodes[c]:
                    t_fin = s + int(deg[nd]) - 1
                    fc[t_fin] = fc.get(t_fin, 0) + 1
                if any(fin_counts[c].get(t, 0) + k > EXT_K for t, k in fc.items()):
                    ok = False
                    break
            if ok:
                break
            s += 1
        for c in range(NCORES):
            for nd in gen_nodes[c]:
                t_fin = s + int(deg[nd]) - 1
                fin_counts[c][t_fin] = fin_counts[c].get(t_fin, 0) + 1
            qpos[c] += len(gen_nodes[c])
        generations.append((g, s, L, gen_nodes))
        next_free[g] = s + L

    S = max(next_free)

    gidx = np.zeros((NCORES, S, NCOL), np.int16)
    eidx = np.zeros((NCORES, S, EXT_K), np.int16)
    ecnt = np.zeros((NCORES, S), np.int32)
    ext_node = [dict() for _ in range(NCORES)]
    resets = sorted({(s - 1, g) for (g, s, L, _) in generations if s > 0})
    for (g, s, L, gen_nodes) in generations:
        col0 = g * GSIZE
        for c in range(NCORES):
            for j, nd in enumerate(gen_nodes[c]):
                d_ = int(deg[nd])
                st = int(starts[nd])
                col = col0 + j
                gidx[c, s:s + d_, col] = edge_trg[st:st + d_]
                t_fin = s + d_ - 1
                k = ecnt[c, t_fin]
                eidx[c, t_fin, k] = col
                ext_node[c][t_fin * EXT_K + int(k)] = nd
                ecnt[c, t_fin] += 1

    zero_slot = S * EXT_K
    cidx = np.zeros((NCORES, NPROJ), np.int16)
    pidx = np.zeros((NCORES, NPROJ), np.int16)
    row_node = np.full((NCORES, NPROJ), -1, np.int64)
    for c in range(NCORES):
        r = 0
        for slot in sorted(ext_node[c].keys()):
            nd = ext_node[c][slot]
            assert slot < 32768
            cidx[c, r] = slot
            pidx[c, r] = nd
            row_node[c, r] = nd
            r += 1
        for nd in iso[c]:
            cidx[c, r] = zero_slot
            pidx[c, r] = nd
            row_node[c, r] = nd
            r += 1
        assert r == NPER
    return dict(S=S, gidx=gidx, eidx=eidx, cidx=cidx, pidx=pidx,
                row_node=row_node, resets=resets)


def _wrap_idx16(idx):
    """[..., n] -> [..., 128, n//16] int16 wrapped+replicated gather layout."""
    idx = np.asarray(idx, np.int16)
    n = idx.shape[-1]
    assert n % 16 == 0
    cols = n // 16
    base = np.swapaxes(idx.reshape(idx.shape[:-1] + (cols, 16)), -1, -2)
    return np.broadcast_to(
        base[..., None, :, :],
        idx.shape[:-1] + (8, 16, cols),
    ).reshape(idx.shape[:-1] + (128, cols))


# --------------------------------------------------------------------------
# device program
# --------------------------------------------------------------------------

def _build_program(S, resets, repeat=1):
    nc = bacc.Bacc("TRN2", target_bir_lowering=False, debug=False)
    x16 = nc.dram_tensor("x16", [N_NODES, D], F16, kind="ExternalInput")
    x32 = nc.dram_tensor("x32", [N_NODES, D], F32, kind="ExternalInput")
    wih = nc.dram_tensor("wih", [D, 4 * HID], F16, kind="ExternalInput")
    whh = nc.dram_tensor("whh", [HID, 4 * HID], F32, kind="ExternalInput")
    bias = nc.dram_tensor("bias", [HID, 4], F32, kind="ExternalInput")
    woutx = nc.dram_tensor("woutx", [D, D], F32, kind="ExternalInput")
    wouth = nc.dram_tensor("wouth", [HID, D], F32, kind="ExternalInput")
    ident = nc.dram_tensor("ident", [128, 128], F32, kind="ExternalInput")
    gidx = nc.dram_tensor("gidx", [128, S * (NCOL // 16)], I16, kind="ExternalInput")
    eidx = nc.dram_tensor("eidx", [128, S * (EXT_K // 16)], I16, kind="ExternalInput")
    cidx = nc.dram_tensor("cidx", [128, NPROJ // 16], I16, kind="ExternalInput")
    pidx = nc.dram_tensor("pidx", [128, NPROJ // 16], I16, kind="ExternalInput")
    out_d = nc.dram_tensor("out", [NPROJ, D], F32, kind="ExternalOutput")

    resets_by_step = {}
    for (t, g) in resets:
        resets_by_step.setdefault(t, []).append(g)

    AGG_COLS = S * EXT_K + GSIZE  # + reserved zero region

    with tile.TileContext(nc) as tc:
        with ExitStack() as ctx:
            sing = ctx.enter_context(tc.tile_pool(name="sing", bufs=1))
            xpool = ctx.enter_context(tc.tile_pool(name="xp", bufs=4))
            apool = ctx.enter_context(tc.tile_pool(name="ap", bufs=2))

            # persistent state + constants
            h_t = sing.tile([128, NCOL], F32R)
            c_t = sing.tile([128, NCOL], F16)
            agg_t = sing.tile([128, AGG_COLS], F32)
            wih_t = sing.tile([D, 4 * HID], F16)
            whh_s = sing.tile([HID, 4 * HID], F32)
            whh_t = sing.tile([HID, 4 * HID], F32R)
            bias_t = sing.tile([HID, 4], F32)
            woutx_t = sing.tile([D, D], F32)
            wouth_t = sing.tile([HID, D], F32)
            ident_t = sing.tile([128, 128], F32)
            gidx_t = sing.tile([128, S * (NCOL // 16)], I16)
            eidx_t = sing.tile([128, S * (EXT_K // 16)], I16)
            cidx_t = sing.tile([128, NPROJ // 16], I16)
            pidx_t = sing.tile([128, NPROJ // 16], I16)

            nc.sync.dma_start(out=wih_t, in_=wih[:, :])
            nc.sync.dma_start(out=whh_s, in_=whh[:, :])
            nc.sync.dma_start(out=bias_t, in_=bias[:, :])
            nc.sync.dma_start(out=woutx_t, in_=woutx[:, :])
            nc.sync.dma_start(out=wouth_t, in_=wouth[:, :])
            nc.sync.dma_start(out=ident_t, in_=ident[:, :])
            nc.sync.dma_start(out=gidx_t, in_=gidx[:, :])
            nc.sync.dma_start(out=eidx_t, in_=eidx[:, :])
            nc.sync.dma_start(out=cidx_t, in_=cidx[:, :])
            nc.sync.dma_start(out=pidx_t, in_=pidx[:, :])

            nc.vector.tensor_copy(whh_t, whh_s)
            nc.vector.memset(h_t.bitcast(mybir.dt.uint32), 0)
            nc.vector.memset(c_t, 0.0)
            nc.vector.memset(agg_t[:, S * EXT_K:], 0.0)

            SIG = mybir.ActivationFunctionType.Sigmoid
            TANH = mybir.ActivationFunctionType.Tanh

            psum_ctx = ExitStack()
            psum = psum_ctx.enter_context(
                tc.tile_pool(name="ps", bufs=1, space="PSUM"))

            def stream_step(t, ss):
                """One LSTM step for stream ss (columns ss*512 .. ss*512+512)."""
                sl = slice(ss * 512, ss * 512 + 512)
                i0 = t * (NCOL // 16) + ss * 32
                xT = xpool.tile([128, 1, 512], F16, name=f"xT{ss}", tag=f"xT{ss}")
                nc.gpsimd.dma_gather(
                    out_ap=xT[:, :, :],
                    in_ap=x16[:, :],
                    idxs_ap=gidx_t[:, i0:i0 + 32],
                    num_idxs=512,
                    num_idxs_reg=512,
                    elem_size=D,
                    transpose=True,
                )
                gates = [psum.tile([128, 512], F32, name=f"g{k}s{ss}",
                                   tag=f"g{k}s{ss}") for k in range(4)]
                for k in range(4):
                    # x part (fp16, 1 cyc/row) + h part (fp32r, 1 cyc/row);
                    # fp32 would cost 4 cycles/row on the PE
                    nc.tensor.matmul(gates[k], wih_t[:, k * HID:(k + 1) * HID],
                                     xT[:, 0, :], start=True, stop=False)
                    nc.tensor.matmul(gates[k], whh_t[:, k * HID:(k + 1) * HID],
                                     h_t[:, sl], start=False, stop=True)
                sig_i = apool.tile([128, 512], F16, name=f"si{ss}", tag=f"si{ss}")
                sig_f = apool.tile([128, 512], F16, name=f"sf{ss}", tag=f"sf{ss}")
                sig_o = apool.tile([128, 512], F16, name=f"so{ss}", tag=f"so{ss}")
                tanh_g = apool.tile([128, 512], F16, name=f"tg{ss}", tag=f"tg{ss}")
                tanh_c = apool.tile([128, 512], F16, name=f"tc{ss}", tag=f"tc{ss}")
                tmp = apool.tile([128, 512], F16, name=f"tmp{ss}", tag=f"tmp{ss}")
                nc.scalar.activation(out=sig_f, in_=gates[1][:, :], func=SIG,
                                     bias=bias_t[:, 1:2])
                nc.scalar.activation(out=sig_i, in_=gates[0][:, :], func=SIG,
                                     bias=bias_t[:, 0:1])
                nc.scalar.activation(out=tanh_g, in_=gates[3][:, :], func=TANH,
                                     bias=bias_t[:, 3:4])
                nc.vector.tensor_mul(c_t[:, sl], sig_f, c_t[:, sl])
                nc.vector.tensor_mul(tmp, sig_i, tanh_g)
                nc.scalar.activation(out=sig_o, in_=gates[2][:, :], func=SIG,
                                     bias=bias_t[:, 2:3])
                nc.vector.tensor_add(c_t[:, sl], c_t[:, sl], tmp)
                nc.scalar.activation(out=tanh_c, in_=c_t[:, sl], func=TANH)
                nc.vector.tensor_mul(h_t[:, sl], sig_o, tanh_c)

            for _rep in range(repeat):
             for t in range(S):
                stream_step(t, 0)
                stream_step(t, 1)
                nc.gpsimd.ap_gather(
                    out_ap=agg_t.bitcast(F32)[:, t * EXT_K:(t + 1) * EXT_K],
                    in_ap=h_t.bitcast(F32)[:, :],
                    idxs_ap=eidx_t[:, t * (EXT_K // 16):(t + 1) * (EXT_K // 16)],
                    channels=128,
                    num_elems=NCOL,
                    d=1,
                    num_idxs=EXT_K,
                )
                for g in resets_by_step.get(t, []):
                    nc.vector.memset(h_t[:, g * GSIZE:(g + 1) * GSIZE].bitcast(mybir.dt.uint32), 0)
                    nc.vector.memset(c_t[:, g * GSIZE:(g + 1) * GSIZE], 0.0)

            # ---- compaction + projection ----
            psum_ctx.close()
            ppsum = ctx.enter_context(
                tc.tile_pool(name="pps", bufs=2, space="PSUM"))
            aggc = sing.tile([128, NPROJ], F32)
            for c0 in range(0, NPROJ, 512):
                w = min(512, NPROJ - c0)
                nc.gpsimd.ap_gather(
                    out_ap=aggc.bitcast(F32)[:, c0:c0 + w],
                    in_ap=agg_t.bitcast(F32)[:, :],
                    idxs_ap=cidx_t[:, c0 // 16:(c0 + w) // 16],
                    channels=128,
                    num_elems=AGG_COLS,
                    d=1,
                    num_idxs=w,
                )
            xrows = sing.tile([128, PTILES, D], F32)
            for c0 in range(0, NPROJ, 512):
                w = min(512, NPROJ - c0)
                nc.gpsimd.dma_gather(
                    out_ap=xrows[:, c0 // 128:(c0 + w) // 128, :],
                    in_ap=x32[:, :],
                    idxs_ap=pidx_t[:, c0 // 16:(c0 + w) // 16],
                    num_idxs=w,
                    num_idxs_reg=w,
                    elem_size=D,
                    transpose=False,
                )
            for s in range(PTILES):
                tp = ppsum.tile([128, 128], F32, tag="tp")
                nc.tensor.transpose(tp, xrows[:, s, :], ident_t)
                xTp = apool.tile([128, 128], F32, tag="xTp")
                nc.vector.tensor_copy(xTp, tp)
                op = ppsum.tile([128, 128], F32, tag="op")
                nc.tensor.matmul(op, xTp, woutx_t, start=True, stop=False)
                nc.tensor.matmul(op, aggc[:, s * 128:(s + 1) * 128], wouth_t,
                                 start=False, stop=True)
                outp = apool.tile([128, 128], F32, tag="outp")
                nc.vector.tensor_copy(outp, op)
                nc.sync.dma_start(out=out_d[s * 128:(s + 1) * 128, :], in_=outp)
    nc.finalize()
    return nc


# --------------------------------------------------------------------------
# entry point
# --------------------------------------------------------------------------

def _prepare(input_matrix, W_ih, W_hh, b_ih, b_hh, W_out,
             edge_src_idxs, edge_trg_idxs):
    sch = _build_schedule(np.asarray(edge_src_idxs, np.int64),
                          np.asarray(edge_trg_idxs, np.int64))
    S = sch["S"]
    nc = _build_program(S, sch["resets"])

    perm = [0, 1, 3, 2]  # device gate order: i, f, o, g
    b = (np.asarray(b_ih) + np.asarray(b_hh)).astype(np.float32)
    W_ih = np.asarray(W_ih, np.float32)
    W_hh = np.asarray(W_hh, np.float32)
    wih_host = np.concatenate(
        [W_ih[p * HID:(p + 1) * HID].T for p in perm], axis=1).astype(np.float16)
    whh_host = np.concatenate(
        [W_hh[p * HID:(p + 1) * HID].T for p in perm], axis=1).astype(np.float32)
    bias_host = np.stack([b[p * HID:(p + 1) * HID] for p in perm], axis=1)
    W_out = np.asarray(W_out, np.float32)
    x32 = np.ascontiguousarray(np.asarray(input_matrix, np.float32))
    x16 = x32.astype(np.float16)
    ident = np.eye(128, dtype=np.float32)

    in_maps = []
    for c in range(NCORES):
        in_maps.append({
            "x16": x16,
            "x32": x32,
            "wih": wih_host,
            "whh": whh_host,
            "bias": bias_host,
            "woutx": np.ascontiguousarray(W_out[:D]),
            "wouth": np.ascontiguousarray(W_out[D:]),
            "ident": ident,
            "gidx": np.ascontiguousarray(
                _wrap_idx16(sch["gidx"][c]).transpose(1, 0, 2).reshape(128, -1)),
            "eidx": np.ascontiguousarray(
                _wrap_idx16(sch["eidx"][c]).transpose(1, 0, 2).reshape(128, -1)),
            "cidx": _wrap_idx16(sch["cidx"][c]),
            "pidx": _wrap_idx16(sch["pidx"][c]),
        })
    return nc, in_maps, sch


def kernel(input_matrix, W_ih, W_hh, b_ih, b_hh, W_out,
           edge_src_idxs, edge_trg_idxs, max_deg, _trace=False):
    nc, in_maps, sch = _prepare(input_matrix, W_ih, W_hh, b_ih, b_hh, W_out,
                                edge_src_idxs, edge_trg_idxs)
    res = run_bass_kernel_spmd(nc, in_maps, core_ids=list(range(NCORES)),
                               trace=_trace)
    out = np.zeros((N_NODES, D), np.float32)
    for c in range(NCORES):
        rows = res.results[c]["out"]           # [NPROJ, 128]
        valid = sch["row_node"][c] >= 0
        out[sch["row_node"][c][valid]] = rows[valid]
    kernel._last_exec_time_ns = res.exec_time_ns
    kernel._last_result = res
    return out

